# revision 9
# baseline (speedup 1.0000x reference)
"""Multi-head attention (B=8, N=1024, C=768, H=12) on 8 TRN2 NeuronCores.

Data-parallel: one batch element per core. Inside each core everything is
kept feature-major (transposed) so no on-chip transposes are needed:

  qkT  = [w_q * scale ; w_k] @ x^T          -> [1536, 1024]  (feature-major)
  V    = x @ w_v^T                          -> [1024, 768]   (token-major)
  S^T  = K_h @ Q_h^T                        -> [1024m, 1024n] per head
  P^T  = exp(S^T)            (no max-sub: scores ~ N(0,1), fp32-safe)
  [out_h ; Z] = [V_h | 1]^T @ P^T           -> [65, n]  (Z = softmax denom)
  attn^T[h] = out_h * (1/Z broadcast via K=1 outer-product matmul)
  y^T  = w_proj @ attn^T + b                -> [768, 1024]

Matmuls run as float32r (f32 storage, full-rate PE mode).
"""

import sys

if "/opt/trn_rl_repo" not in sys.path:
    sys.path.insert(0, "/opt/trn_rl_repo")

import numpy as np

import concourse.bass as bass  # noqa: F401
import concourse.mybir as mybir
import concourse.tile as tile
from concourse import bacc
from concourse.bass_utils import run_bass_kernel_spmd

F32 = mybir.dt.float32
F32R = mybir.dt.float32r
AF = mybir.ActivationFunctionType

B, N, C = 8, 1024, 768
H, D = 12, 64
SCALE = D ** -0.5
KT = C // 128           # 6 contraction tiles
NT = N // 128           # 8 token tiles
NCH = N // 512          # 2 free-dim chunks of 512

_CACHE = {}


def _r(ap):
    return ap


def build():
    nc = bacc.Bacc("TRN2", target_bir_lowering=False, debug=False, num_devices=8)

    xT_d = nc.dram_tensor("xT", [C, N], F32, kind="ExternalInput")
    wqk_d = nc.dram_tensor("w_qkT", [C, 2 * C], F32, kind="ExternalInput")
    wv_d = nc.dram_tensor("w_vT", [C, C], F32, kind="ExternalInput")
    wp_d = nc.dram_tensor("w_pT", [C, C], F32, kind="ExternalInput")
    b_d = nc.dram_tensor("b_p", [C, 1], F32, kind="ExternalInput")
    out_d = nc.dram_tensor("out", [C, N], F32, kind="ExternalOutput")

    with tile.TileContext(nc) as tc:
        _body(nc, tc, xT_d, wqk_d, wv_d, wp_d, b_d, out_d)
    nc.compile()
    return nc


def _body(nc, tc, xT_d, wqk_d, wv_d, wp_d, b_d, out_d):
    from contextlib import ExitStack

    with ExitStack() as ctx:
        ctx.enter_context(
            nc.allow_low_precision(reason="f32r matmul inputs; accum stays f32")
        )
        const = ctx.enter_context(tc.tile_pool(name="const", bufs=1))
        qk_pool = ctx.enter_context(tc.tile_pool(name="qk", bufs=1))
        v_pool = ctx.enter_context(tc.tile_pool(name="v", bufs=1))

        ones_raw = const.tile([1, 64], F32)
        nc.vector.memset(ones_raw, 1.0)
        ones_col = const.tile([1, 64], F32R)
        nc.vector.tensor_copy(ones_col, ones_raw)
        ones12 = const.tile([128, 12], F32)
        nc.vector.memset(ones12, 1.0)
        b_sb = const.tile([128, KT], F32)
        for ot in range(KT):
            nc.sync.dma_start(
                out=b_sb[:, ot : ot + 1], in_=b_d.ap()[ot * 128 : (ot + 1) * 128, :]
            )

        qkT = [qk_pool.tile([128, N], F32R, tag=f"qkT{i}", name=f"qkT{i}") for i in range(12)]
        v_sb = [v_pool.tile([128, H, 65], F32R, tag=f"v{i}", name=f"v{i}") for i in range(NT)]

        # ---------------- stage 1+2+3: load inputs, QK^T, V ----------------
        with ExitStack() as sctx:
            x_pool = sctx.enter_context(tc.tile_pool(name="x", bufs=1))
            wqk_pool = sctx.enter_context(tc.tile_pool(name="wqk", bufs=1))
            wv_pool = sctx.enter_context(tc.tile_pool(name="wv", bufs=1))
            ps_pool = sctx.enter_context(
                tc.tile_pool(name="ps_qkv", bufs=4, space="PSUM")
            )

            raw_pool = sctx.enter_context(tc.tile_pool(name="raw", bufs=4))
            xT = [x_pool.tile([128, N], F32R, tag=f"x{i}", name=f"x{i}") for i in range(KT)]
            wqk = [wqk_pool.tile([128, 2 * C], F32R, tag=f"wqk{i}", name=f"wqk{i}") for i in range(KT)]
            wv = [wv_pool.tile([128, C], F32R, tag=f"wv{i}", name=f"wv{i}") for i in range(KT)]
            for kt in range(KT):
                ksl = slice(kt * 128, (kt + 1) * 128)
                for dram, dst in ((xT_d, xT[kt]), (wqk_d, wqk[kt]), (wv_d, wv[kt])):
                    raw = raw_pool.tile(dst.shape, F32, tag="raw", name=f"raw_{dram.name}_{kt}")
                    nc.sync.dma_start(out=raw, in_=dram.ap()[ksl, :])
                    nc.vector.tensor_copy(dst, raw)

            # qkT[ot] = sum_kt wqk[kt][:, ot]^T @ xT[kt]   (feature-major q,k)
            for ot in range(12):
                osl = slice(ot * 128, (ot + 1) * 128)
                for nch in range(NCH):
                    nsl = slice(nch * 512, (nch + 1) * 512)
                    ps = ps_pool.tile([128, 512], F32, tag="ps_qkv")
                    for kt in range(KT):
                        nc.tensor.matmul(
                            ps,
                            lhsT=_r(wqk[kt][:, osl]),
                            rhs=_r(xT[kt][:, nsl]),
                            start=(kt == 0),
                            stop=(kt == KT - 1),
                        )
                    nc.scalar.copy(out=qkT[ot][:, nsl], in_=ps)

            # V token-major with per-head ones column (65-stride layout)
            for mt in range(NT):
                msl = slice(mt * 128, (mt + 1) * 128)
                nc.vector.tensor_copy(v_sb[mt][:, :, 64:65], ones12.unsqueeze(-1))
                for o0, ow in ((0, 512), (512, 256)):
                    nh = ow // 64
                    ps = ps_pool.tile([128, 512], F32, tag="ps_qkv")
                    for kt in range(KT):
                        nc.tensor.matmul(
                            ps[:, :ow],
                            lhsT=_r(xT[kt][:, msl]),
                            rhs=_r(wv[kt][:, o0 : o0 + ow]),
                            start=(kt == 0),
                            stop=(kt == KT - 1),
                        )
                    dst = v_sb[mt][:, o0 // 64 : o0 // 64 + nh, 0:64]
                    src = ps[:, :ow].rearrange("p (h e) -> p h e", e=64)
                    nc.vector.tensor_copy(dst, src)

        # ---------------- stage 4: per-head attention ----------------
        with ExitStack() as sctx:
            attn_pool = sctx.enter_context(tc.tile_pool(name="attn", bufs=1))
            wp_pool = sctx.enter_context(tc.tile_pool(name="wp", bufs=1))
            pt_pool = sctx.enter_context(tc.tile_pool(name="pt", bufs=4))
            zr_pool = sctx.enter_context(tc.tile_pool(name="zr", bufs=4))
            y_pool = sctx.enter_context(tc.tile_pool(name="y", bufs=3))
            ps_s = sctx.enter_context(tc.tile_pool(name="ps_s", bufs=2, space="PSUM"))
            ps_av = sctx.enter_context(tc.tile_pool(name="ps_av", bufs=2, space="PSUM"))

            attnT = [attn_pool.tile([128, N], F32R, tag=f"at{i}", name=f"at{i}") for i in range(KT)]
            raw2_pool = sctx.enter_context(tc.tile_pool(name="raw2", bufs=2))
            wp = [wp_pool.tile([128, C], F32R, tag=f"wp{i}", name=f"wp{i}") for i in range(KT)]
            for kt in range(KT):
                raw = raw2_pool.tile([128, C], F32, tag="raw2", name=f"raw_wp_{kt}")
                nc.sync.dma_start(out=raw, in_=wp_d.ap()[kt * 128 : (kt + 1) * 128, :])
                nc.vector.tensor_copy(wp[kt], raw)

            GM = 2  # m-tiles per scores/exp group
            for hp in range(H // 2):  # head pairs (row-group packing)
                q_t = qkT[hp]
                k_t = qkT[6 + hp]
                for nch in range(NCH):
                    nsl = slice(nch * 512, (nch + 1) * 512)
                    pav = [
                        ps_av.tile([65, 512], F32, tag="ps_av", name=f"pav0_{hp}_{nch}"),
                        ps_av.tile([65, 512], F32, tag="ps_av", name=f"pav1_{hp}_{nch}"),
                    ]
                    for g in range(NT // GM):  # groups of GM m-tiles
                        pss = [
                            ps_s.tile([128, GM * 512], F32, tag="ps_s", name=f"pss0_{hp}_{nch}_{g}"),
                            ps_s.tile([128, GM * 512], F32, tag="ps_s", name=f"pss1_{hp}_{nch}_{g}"),
                        ]
                        for j in range(GM):
                            mt = g * GM + j
                            msl = slice(mt * 128, (mt + 1) * 128)
                            jsl = slice(j * 512, (j + 1) * 512)
                            for e in range(2):  # even/odd head of the pair
                                esl = slice(e * 64, e * 64 + 64)
                                nc.tensor.matmul(
                                    pss[e][:, jsl],
                                    lhsT=_r(k_t[esl, msl]),
                                    rhs=_r(q_t[esl, nsl]),
                                    start=True,
                                    stop=True,
                                )
                        pt = [
                            pt_pool.tile([128, GM * 512], F32R, tag="pt", name=f"pt0_{hp}_{nch}_{g}"),
                            pt_pool.tile([128, GM * 512], F32R, tag="pt", name=f"pt1_{hp}_{nch}_{g}"),
                        ]
                        nc.scalar.activation(pt[0], pss[0], AF.Exp)
                        nc.scalar.activation(pt[1], pss[1], AF.Exp)
                        for j in range(GM):
                            mt = g * GM + j
                            jsl = slice(j * 512, (j + 1) * 512)
                            for e in range(2):
                                h = 2 * hp + e
                                nc.tensor.matmul(
                                    pav[e],
                                    lhsT=_r(v_sb[mt][:, h, :]),
                                    rhs=_r(pt[e][:, jsl]),
                                    start=(mt == 0),
                                    stop=(mt == NT - 1),
                                )
                    for e in range(2):
                        h = 2 * hp + e
                        zr = zr_pool.tile([1, 512], F32R, tag="zr")
                        nc.vector.reciprocal(zr, pav[e][64:65, :])
                        zb = ps_av.tile([64, 512], F32, tag="misc", name=f"zb_{hp}_{nch}_{e}")
                        nc.tensor.matmul(
                            zb, lhsT=_r(ones_col), rhs=_r(zr), start=True, stop=True
                        )
                        dst = attnT[h // 2][(h % 2) * 64 : (h % 2) * 64 + 64, nsl]
                        # two PSUM reads in one op are illegal: stage AV in
                        # SBUF (rounds to f32r), then scale by 1/Z in place
                        nc.vector.tensor_copy(dst, pav[e][0:64, :])
                        nc.vector.tensor_mul(dst, dst, zb)

            # ---------------- stage 5: output projection ----------------
            for ot in range(KT):
                osl = slice(ot * 128, (ot + 1) * 128)
                for nch in range(NCH):
                    nsl = slice(nch * 512, (nch + 1) * 512)
                    ps = ps_av.tile([128, 512], F32, tag="misc", name=f"ps_y_{ot}_{nch}")
                    for kt in range(KT):
                        nc.tensor.matmul(
                            ps,
                            lhsT=_r(wp[kt][:, osl]),
                            rhs=_r(attnT[kt][:, nsl]),
                            start=(kt == 0),
                            stop=(kt == KT - 1),
                        )
                    y = y_pool.tile([128, 512], F32, tag="y")
                    nc.scalar.activation(y, ps, AF.Identity, bias=b_sb[:, ot : ot + 1])
                    nc.sync.dma_start(out=out_d.ap()[osl, nsl], in_=y)


def _get_nc():
    if "nc" not in _CACHE:
        _CACHE["nc"] = build()
    return _CACHE["nc"]


def kernel(x, w_qkv, w_proj, b_proj, _trace=False):
    x = np.asarray(x, dtype=np.float32)
    w_qkv = np.asarray(w_qkv, dtype=np.float32)
    w_proj = np.asarray(w_proj, dtype=np.float32)
    b_proj = np.asarray(b_proj, dtype=np.float32)

    wq = w_qkv[0:C] * np.float32(SCALE)
    wk = w_qkv[C : 2 * C]
    wv = w_qkv[2 * C : 3 * C]
    w_qkT = np.ascontiguousarray(np.concatenate([wq, wk], axis=0).T)  # [C, 2C]
    w_vT = np.ascontiguousarray(wv.T)  # [C, C]
    w_pT = np.ascontiguousarray(w_proj.T)  # [C, C]
    b_p = np.ascontiguousarray(b_proj.reshape(C, 1))

    in_maps = []
    for i in range(B):
        in_maps.append(
            {
                "xT": np.ascontiguousarray(x[i].T),  # [C, N]
                "w_qkT": w_qkT,
                "w_vT": w_vT,
                "w_pT": w_pT,
                "b_p": b_p,
            }
        )

    nc = _get_nc()
    res = run_bass_kernel_spmd(nc, in_maps, core_ids=list(range(B)), trace=_trace)
    _CACHE["last_result"] = res

    out = np.empty((B, N, C), dtype=np.float32)
    for i in range(B):
        out[i] = res.results[i]["out"].T
    return out


# revision 11
# speedup vs baseline: 1.4059x; 1.4059x over previous
"""Multi-head attention (B=8, N=1024, C=768, H=12) on 8 TRN2 NeuronCores.

Data-parallel: one batch element per core. Inside each core everything is
kept feature-major (transposed) so no on-chip transposes are needed:

  qkT  = [w_q * scale ; w_k] @ x^T          -> [1536, 1024]  (feature-major)
  V    = x @ w_v^T                          -> [1024, 768]   (token-major)
  S^T  = K_h @ Q_h^T                        -> [1024m, 1024n] per head
  P^T  = exp(S^T)            (no max-sub: scores ~ N(0,1), fp32-safe)
  [out_h ; Z] = [V_h | 1]^T @ P^T           -> [65, n]  (Z = softmax denom)
  attn^T[h] = out_h / (Z broadcast via K=1 outer-product matmul)
  y^T  = w_proj @ attn^T + b                -> [768, 1024]

Matmul operands are bf16 (fp32 PSUM accumulation); the softmax-denominator
broadcast runs in float32r to keep Z at ~fp32 precision.
"""

import sys

if "/opt/trn_rl_repo" not in sys.path:
    sys.path.insert(0, "/opt/trn_rl_repo")

import numpy as np

import concourse.bass as bass  # noqa: F401
import concourse.mybir as mybir
import concourse.tile as tile
from concourse import bacc
from concourse.bass_utils import run_bass_kernel_spmd

F32 = mybir.dt.float32
F32R = mybir.dt.float32r
BF16 = mybir.dt.bfloat16
AF = mybir.ActivationFunctionType
ALU = mybir.AluOpType

B, N, C = 8, 1024, 768
H, D = 12, 64
SCALE = D ** -0.5
KT = C // 128           # 6 contraction tiles
NT = N // 128           # 8 token tiles
NCH = N // 512          # 2 free-dim chunks of 512

_CACHE = {}


def build():
    nc = bacc.Bacc("TRN2", target_bir_lowering=False, debug=False, num_devices=8)

    xT_d = nc.dram_tensor("xT", [C, N], F32, kind="ExternalInput")
    wqk_d = nc.dram_tensor("w_qkT", [C, 2 * C], F32, kind="ExternalInput")
    wv_d = nc.dram_tensor("w_vT", [C, C], F32, kind="ExternalInput")
    wp_d = nc.dram_tensor("w_pT", [C, C], F32, kind="ExternalInput")
    b_d = nc.dram_tensor("b_p", [C, 1], F32, kind="ExternalInput")
    out_d = nc.dram_tensor("out", [C, N], F32, kind="ExternalOutput")

    with tile.TileContext(nc) as tc:
        _body(nc, tc, xT_d, wqk_d, wv_d, wp_d, b_d, out_d)
    nc.compile()
    return nc


def _body(nc, tc, xT_d, wqk_d, wv_d, wp_d, b_d, out_d):
    from contextlib import ExitStack

    with ExitStack() as ctx:
        ctx.enter_context(
            nc.allow_low_precision(reason="bf16 matmul operands; accum stays f32")
        )
        const = ctx.enter_context(tc.tile_pool(name="const", bufs=1))
        qk_pool = ctx.enter_context(tc.tile_pool(name="qk", bufs=1))
        v_pool = ctx.enter_context(tc.tile_pool(name="v", bufs=1))

        ones_raw = const.tile([1, 64], F32)
        nc.vector.memset(ones_raw, 1.0)
        ones_col = const.tile([1, 64], F32R)
        nc.vector.tensor_copy(ones_col, ones_raw)
        ones12 = const.tile([128, 12], F32)
        nc.vector.memset(ones12, 1.0)
        b_sb = const.tile([128, KT], F32)
        for ot in range(KT):
            nc.sync.dma_start(
                out=b_sb[:, ot : ot + 1], in_=b_d.ap()[ot * 128 : (ot + 1) * 128, :]
            )

        qkT = [qk_pool.tile([128, N], BF16, tag=f"qkT{i}", name=f"qkT{i}") for i in range(12)]
        v_sb = [v_pool.tile([128, H, 65], BF16, tag=f"v{i}", name=f"v{i}") for i in range(NT)]

        # ---------------- stage 1+2+3: load inputs, QK^T, V ----------------
        with ExitStack() as sctx:
            x_pool = sctx.enter_context(tc.tile_pool(name="x", bufs=1))
            wqk_pool = sctx.enter_context(tc.tile_pool(name="wqk", bufs=1))
            wv_pool = sctx.enter_context(tc.tile_pool(name="wv", bufs=1))
            ps_pool = sctx.enter_context(
                tc.tile_pool(name="ps_qkv", bufs=4, space="PSUM")
            )

            raw_pool = sctx.enter_context(tc.tile_pool(name="raw", bufs=4))
            xT = [x_pool.tile([128, N], BF16, tag=f"x{i}", name=f"x{i}") for i in range(KT)]
            wqk = [wqk_pool.tile([128, 2 * C], BF16, tag=f"wqk{i}", name=f"wqk{i}") for i in range(KT)]
            wv = [wv_pool.tile([128, C], BF16, tag=f"wv{i}", name=f"wv{i}") for i in range(KT)]
            for kt in range(KT):
                ksl = slice(kt * 128, (kt + 1) * 128)
                for dram, dst in ((xT_d, xT[kt]), (wqk_d, wqk[kt]), (wv_d, wv[kt])):
                    raw = raw_pool.tile(dst.shape, F32, tag="raw", name=f"raw_{dram.name}_{kt}")
                    nc.sync.dma_start(out=raw, in_=dram.ap()[ksl, :])
                    nc.vector.tensor_copy(dst, raw)

            # qkT[ot] = sum_kt wqk[kt][:, ot]^T @ xT[kt]   (feature-major q,k)
            for ot in range(12):
                osl = slice(ot * 128, (ot + 1) * 128)
                for nch in range(NCH):
                    nsl = slice(nch * 512, (nch + 1) * 512)
                    ps = ps_pool.tile([128, 512], F32, tag="ps_qkv")
                    for kt in range(KT):
                        nc.tensor.matmul(
                            ps,
                            lhsT=wqk[kt][:, osl],
                            rhs=xT[kt][:, nsl],
                            start=(kt == 0),
                            stop=(kt == KT - 1),
                        )
                    nc.vector.tensor_copy(qkT[ot][:, nsl], ps)

            # V token-major with per-head ones column (65-stride layout)
            for mt in range(NT):
                msl = slice(mt * 128, (mt + 1) * 128)
                nc.vector.tensor_copy(v_sb[mt][:, :, 64:65], ones12.unsqueeze(-1))
                for o0, ow in ((0, 512), (512, 256)):
                    nh = ow // 64
                    ps = ps_pool.tile([128, 512], F32, tag="ps_qkv")
                    for kt in range(KT):
                        nc.tensor.matmul(
                            ps[:, :ow],
                            lhsT=xT[kt][:, msl],
                            rhs=wv[kt][:, o0 : o0 + ow],
                            start=(kt == 0),
                            stop=(kt == KT - 1),
                        )
                    dst = v_sb[mt][:, o0 // 64 : o0 // 64 + nh, 0:64]
                    src = ps[:, :ow].rearrange("p (h e) -> p h e", e=64)
                    nc.vector.tensor_copy(dst, src)

        # ---------------- stage 4: per-head attention ----------------
        with ExitStack() as sctx:
            attn_pool = sctx.enter_context(tc.tile_pool(name="attn", bufs=1))
            wp_pool = sctx.enter_context(tc.tile_pool(name="wp", bufs=1))
            pt_pool = sctx.enter_context(tc.tile_pool(name="pt", bufs=4))
            zs_pool = sctx.enter_context(tc.tile_pool(name="zs", bufs=4))
            y_pool = sctx.enter_context(tc.tile_pool(name="y", bufs=3))
            ps_s = sctx.enter_context(tc.tile_pool(name="ps_s", bufs=2, space="PSUM"))
            ps_av = sctx.enter_context(tc.tile_pool(name="ps_av", bufs=2, space="PSUM"))

            attnT = [attn_pool.tile([128, N], BF16, tag=f"at{i}", name=f"at{i}") for i in range(KT)]
            raw2_pool = sctx.enter_context(tc.tile_pool(name="raw2", bufs=2))
            wp = [wp_pool.tile([128, C], BF16, tag=f"wp{i}", name=f"wp{i}") for i in range(KT)]
            for kt in range(KT):
                raw = raw2_pool.tile([128, C], F32, tag="raw2", name=f"raw_wp_{kt}")
                nc.sync.dma_start(out=raw, in_=wp_d.ap()[kt * 128 : (kt + 1) * 128, :])
                nc.vector.tensor_copy(wp[kt], raw)

            GM = 2  # m-tiles per scores/exp group
            for hp in range(H // 2):  # head pairs (row-group packing)
                q_t = qkT[hp]
                k_t = qkT[6 + hp]
                for nch in range(NCH):
                    nsl = slice(nch * 512, (nch + 1) * 512)
                    pav = [
                        ps_av.tile([65, 512], F32, tag="ps_av", name=f"pav0_{hp}_{nch}"),
                        ps_av.tile([65, 512], F32, tag="ps_av", name=f"pav1_{hp}_{nch}"),
                    ]
                    for g in range(NT // GM):  # groups of GM m-tiles
                        pss = [
                            ps_s.tile([128, GM * 512], F32, tag="ps_s", name=f"pss0_{hp}_{nch}_{g}"),
                            ps_s.tile([128, GM * 512], F32, tag="ps_s", name=f"pss1_{hp}_{nch}_{g}"),
                        ]
                        for j in range(GM):
                            mt = g * GM + j
                            msl = slice(mt * 128, (mt + 1) * 128)
                            jsl = slice(j * 512, (j + 1) * 512)
                            for e in range(2):  # even/odd head of the pair
                                esl = slice(e * 64, e * 64 + 64)
                                nc.tensor.matmul(
                                    pss[e][:, jsl],
                                    lhsT=k_t[esl, msl],
                                    rhs=q_t[esl, nsl],
                                    start=True,
                                    stop=True,
                                )
                        pt = [
                            pt_pool.tile([128, GM * 512], BF16, tag="pt", name=f"pt0_{hp}_{nch}_{g}"),
                            pt_pool.tile([128, GM * 512], BF16, tag="pt", name=f"pt1_{hp}_{nch}_{g}"),
                        ]
                        nc.scalar.activation(pt[0], pss[0], AF.Exp)
                        nc.scalar.activation(pt[1], pss[1], AF.Exp)
                        for j in range(GM):
                            mt = g * GM + j
                            jsl = slice(j * 512, (j + 1) * 512)
                            for e in range(2):
                                h = 2 * hp + e
                                nc.tensor.matmul(
                                    pav[e],
                                    lhsT=v_sb[mt][:, h, :],
                                    rhs=pt[e][:, jsl],
                                    start=(mt == 0),
                                    stop=(mt == NT - 1),
                                )
                    for e in range(2):
                        h = 2 * hp + e
                        # Z lives in row 64 of pav; broadcast it across 64
                        # partitions with a K=1 outer-product matmul (f32r),
                        # take a fast reciprocal on all 64 lanes, then one
                        # fused multiply that reads unnormalized AV from PSUM.
                        z_sb = zs_pool.tile([1, 512], F32R, tag="zs", name=f"z_{hp}_{nch}_{e}")
                        nc.vector.tensor_copy(z_sb, pav[e][64:65, :])
                        zb = ps_av.tile([64, 512], F32, tag="misc", name=f"zb_{hp}_{nch}_{e}")
                        nc.tensor.matmul(
                            zb, lhsT=ones_col, rhs=z_sb, start=True, stop=True
                        )
                        zr64 = zs_pool.tile([64, 512], F32, tag="zr64", name=f"zr64_{hp}_{nch}_{e}")
                        nc.vector.reciprocal_approx_fast(out=zr64, in_=zb)
                        dst = attnT[h // 2][(h % 2) * 64 : (h % 2) * 64 + 64, nsl]
                        nc.vector.tensor_mul(dst, zr64, pav[e][0:64, :])

            # ---------------- stage 5: output projection ----------------
            for ot in range(KT):
                osl = slice(ot * 128, (ot + 1) * 128)
                for nch in range(NCH):
                    nsl = slice(nch * 512, (nch + 1) * 512)
                    ps = ps_av.tile([128, 512], F32, tag="misc", name=f"ps_y_{ot}_{nch}")
                    for kt in range(KT):
                        nc.tensor.matmul(
                            ps,
                            lhsT=wp[kt][:, osl],
                            rhs=attnT[kt][:, nsl],
                            start=(kt == 0),
                            stop=(kt == KT - 1),
                        )
                    y = y_pool.tile([128, 512], F32, tag="y")
                    nc.scalar.activation(y, ps, AF.Identity, bias=b_sb[:, ot : ot + 1])
                    nc.sync.dma_start(out=out_d.ap()[osl, nsl], in_=y)


def _get_nc():
    if "nc" not in _CACHE:
        _CACHE["nc"] = build()
    return _CACHE["nc"]


def kernel(x, w_qkv, w_proj, b_proj, _trace=False):
    x = np.asarray(x, dtype=np.float32)
    w_qkv = np.asarray(w_qkv, dtype=np.float32)
    w_proj = np.asarray(w_proj, dtype=np.float32)
    b_proj = np.asarray(b_proj, dtype=np.float32)

    wq = w_qkv[0:C] * np.float32(SCALE)
    wk = w_qkv[C : 2 * C]
    wv = w_qkv[2 * C : 3 * C]
    w_qkT = np.ascontiguousarray(np.concatenate([wq, wk], axis=0).T)  # [C, 2C]
    w_vT = np.ascontiguousarray(wv.T)  # [C, C]
    w_pT = np.ascontiguousarray(w_proj.T)  # [C, C]
    b_p = np.ascontiguousarray(b_proj.reshape(C, 1))

    in_maps = []
    for i in range(B):
        in_maps.append(
            {
                "xT": np.ascontiguousarray(x[i].T),  # [C, N]
                "w_qkT": w_qkT,
                "w_vT": w_vT,
                "w_pT": w_pT,
                "b_p": b_p,
            }
        )

    nc = _get_nc()
    res = run_bass_kernel_spmd(nc, in_maps, core_ids=list(range(B)), trace=_trace)
    _CACHE["last_result"] = res

    out = np.empty((B, N, C), dtype=np.float32)
    for i in range(B):
        out[i] = res.results[i]["out"].T
    return out


# revision 14
# speedup vs baseline: 1.5747x; 1.1200x over previous
"""Multi-head attention (B=8, N=1024, C=768, H=12) on 8 TRN2 NeuronCores.

Data-parallel: one batch element per core. Inside each core everything is
kept feature-major (transposed) so no on-chip transposes are needed:

  qkT  = [w_q * scale ; w_k] @ x^T          -> [1536, 1024]  (feature-major)
  V    = x @ w_v^T                          -> [1024, 768]   (token-major)
  S^T  = K_h @ Q_h^T                        -> [1024m, 1024n] per head
  P^T  = exp(S^T)            (no max-sub: scores ~ N(0,1), fp32-safe)
  [out_h ; Z] = [V_h | 1]^T @ P^T           -> [65, n]  (Z = softmax denom)
  attn^T[h] = out_h * approx_recip(Z broadcast via K=1 matmul)
  y^T  = w_proj @ attn^T + b                -> [768, 1024]

Matmul operands are bf16 (fp32 PSUM accumulation); the softmax-denominator
path runs in f32/f32r. Head pair 0's scores+exp are emitted during the
QKV stage so ScalarE's exp stream starts early and overlaps PE work.
"""

import sys

if "/opt/trn_rl_repo" not in sys.path:
    sys.path.insert(0, "/opt/trn_rl_repo")

import numpy as np

import concourse.bass as bass  # noqa: F401
import concourse.mybir as mybir
import concourse.tile as tile
from concourse import bacc
from concourse.bass_utils import run_bass_kernel_spmd

F32 = mybir.dt.float32
F32R = mybir.dt.float32r
BF16 = mybir.dt.bfloat16
AF = mybir.ActivationFunctionType

B, N, C = 8, 1024, 768
H, D = 12, 64
SCALE = D ** -0.5
KT = C // 128           # 6 contraction tiles
NT = N // 128           # 8 token tiles
NCH = N // 512          # 2 free-dim chunks of 512
GM = 2                  # m-tiles per scores/exp group

_CACHE = {}


def build():
    nc = bacc.Bacc("TRN2", target_bir_lowering=False, debug=False, num_devices=8)

    xT_d = nc.dram_tensor("xT", [C, N], F32, kind="ExternalInput")
    wqk_d = nc.dram_tensor("w_qkT", [C, 2 * C], F32, kind="ExternalInput")
    wv_d = nc.dram_tensor("w_vT", [C, C], F32, kind="ExternalInput")
    wp_d = nc.dram_tensor("w_pT", [C, C], F32, kind="ExternalInput")
    b_d = nc.dram_tensor("b_p", [C, 1], F32, kind="ExternalInput")
    out_d = nc.dram_tensor("out", [C, N], F32, kind="ExternalOutput")

    with tile.TileContext(nc) as tc:
        _body(nc, tc, xT_d, wqk_d, wv_d, wp_d, b_d, out_d)
    nc.compile()
    return nc


def _scores_exp(nc, qkT, pt_pool, ps_s, hp, nch, tag):
    """Emit scores matmuls + exp for one head pair / n-chunk. Returns the
    exp'd P^T tiles (bf16, [128, GM*512]) for all NT//GM groups x 2 heads."""
    q_t = qkT[hp]
    k_t = qkT[6 + hp]
    nsl = slice(nch * 512, (nch + 1) * 512)
    out = []
    for g in range(NT // GM):
        pss = [
            ps_s.tile([128, GM * 512], F32, tag="ps_s", name=f"pss0_{tag}_{g}"),
            ps_s.tile([128, GM * 512], F32, tag="ps_s", name=f"pss1_{tag}_{g}"),
        ]
        for j in range(GM):
            mt = g * GM + j
            msl = slice(mt * 128, (mt + 1) * 128)
            jsl = slice(j * 512, (j + 1) * 512)
            for e in range(2):  # even/odd head (row-group packed)
                esl = slice(e * 64, e * 64 + 64)
                nc.tensor.matmul(
                    pss[e][:, jsl],
                    lhsT=k_t[esl, msl],
                    rhs=q_t[esl, nsl],
                    start=True,
                    stop=True,
                )
        pt = [
            pt_pool.tile([128, GM * 512], BF16, tag="pt", name=f"pt0_{tag}_{g}"),
            pt_pool.tile([128, GM * 512], BF16, tag="pt", name=f"pt1_{tag}_{g}"),
        ]
        nc.scalar.activation(pt[0], pss[0], AF.Exp)
        nc.scalar.activation(pt[1], pss[1], AF.Exp)
        out.append(pt)
    return out


def _av_norm(nc, v_sb, attnT, pt_tiles, zs_pool, ps_av, ones_col, hp, nch, tag):
    """AV accumulation + softmax normalization for one head pair / n-chunk."""
    nsl = slice(nch * 512, (nch + 1) * 512)
    pav = [
        ps_av.tile([65, 512], F32, tag="ps_av", name=f"pav0_{tag}"),
        ps_av.tile([65, 512], F32, tag="ps_av", name=f"pav1_{tag}"),
    ]
    for g in range(NT // GM):
        pt = pt_tiles[g]
        for j in range(GM):
            mt = g * GM + j
            jsl = slice(j * 512, (j + 1) * 512)
            for e in range(2):
                h = 2 * hp + e
                nc.tensor.matmul(
                    pav[e],
                    lhsT=v_sb[mt][:, h, :],
                    rhs=pt[e][:, jsl],
                    start=(mt == 0),
                    stop=(mt == NT - 1),
                )
    for e in range(2):
        h = 2 * hp + e
        # Stage [out_h ; Z] into SBUF as f32r (releases the PSUM bank),
        # broadcast Z across 64 partitions with a K=1 matmul, take a fast
        # 64-lane reciprocal, then one multiply produces normalized bf16.
        z_sb = zs_pool.tile([1, 512], F32R, tag="z_sb", name=f"z_{tag}_{e}")
        nc.vector.tensor_copy(z_sb, pav[e][64:65, :])
        av_r = zs_pool.tile([64, 512], F32R, tag="av_r", name=f"avr_{tag}_{e}")
        nc.vector.tensor_copy(av_r, pav[e][0:64, :])
        zb = ps_av.tile([64, 512], F32, tag="misc", name=f"zb_{tag}_{e}")
        nc.tensor.matmul(zb, lhsT=ones_col, rhs=z_sb, start=True, stop=True)
        zr64 = zs_pool.tile([64, 512], F32, tag="zr64", name=f"zr_{tag}_{e}")
        nc.vector.reciprocal_approx_fast(out=zr64, in_=zb)
        dst = attnT[h // 2][(h % 2) * 64 : (h % 2) * 64 + 64, nsl]
        nc.vector.tensor_mul(dst, zr64, av_r)


def _body(nc, tc, xT_d, wqk_d, wv_d, wp_d, b_d, out_d):
    from contextlib import ExitStack

    with ExitStack() as ctx:
        ctx.enter_context(
            nc.allow_low_precision(reason="bf16 matmul operands; accum stays f32")
        )
        const = ctx.enter_context(tc.tile_pool(name="const", bufs=1))
        qk_pool = ctx.enter_context(tc.tile_pool(name="qk", bufs=1))
        v_pool = ctx.enter_context(tc.tile_pool(name="v", bufs=1))
        attn_pool = ctx.enter_context(tc.tile_pool(name="attn", bufs=1))
        pt_pool = ctx.enter_context(tc.tile_pool(name="pt", bufs=18))
        zs_pool = ctx.enter_context(tc.tile_pool(name="zs", bufs=4))
        ps_s = ctx.enter_context(tc.tile_pool(name="ps_s", bufs=2, space="PSUM"))

        ones_raw = const.tile([1, 64], F32)
        nc.vector.memset(ones_raw, 1.0)
        ones_col = const.tile([1, 64], F32R)
        nc.vector.tensor_copy(ones_col, ones_raw)
        ones12 = const.tile([128, 12], F32)
        nc.vector.memset(ones12, 1.0)
        b_sb = const.tile([128, KT], F32)
        for ot in range(KT):
            nc.sync.dma_start(
                out=b_sb[:, ot : ot + 1], in_=b_d.ap()[ot * 128 : (ot + 1) * 128, :]
            )

        qkT = [qk_pool.tile([128, N], BF16, tag=f"qkT{i}", name=f"qkT{i}") for i in range(12)]
        v_sb = [v_pool.tile([128, H, 65], BF16, tag=f"v{i}", name=f"v{i}") for i in range(NT)]
        attnT = [attn_pool.tile([128, N], BF16, tag=f"at{i}", name=f"at{i}") for i in range(KT)]

        pt0 = {}  # pair-0 P^T tiles, produced in the QKV scope, consumed later

        # ------- scope A: loads, QK^T, V, plus pair-0 scores+exp -------
        with ExitStack() as sctx:
            x_pool = sctx.enter_context(tc.tile_pool(name="x", bufs=1))
            wqk_pool = sctx.enter_context(tc.tile_pool(name="wqk", bufs=1))
            wv_pool = sctx.enter_context(tc.tile_pool(name="wv", bufs=1))
            raw_pool = sctx.enter_context(tc.tile_pool(name="raw", bufs=4))
            ps_pool = sctx.enter_context(
                tc.tile_pool(name="ps_qkv", bufs=4, space="PSUM")
            )

            xT = [x_pool.tile([128, N], BF16, tag=f"x{i}", name=f"x{i}") for i in range(KT)]
            wqk = [wqk_pool.tile([128, 2 * C], BF16, tag=f"wqk{i}", name=f"wqk{i}") for i in range(KT)]
            wv = [wv_pool.tile([128, C], BF16, tag=f"wv{i}", name=f"wv{i}") for i in range(KT)]
            for kt in range(KT):
                ksl = slice(kt * 128, (kt + 1) * 128)
                for dram, dst, eng in (
                    (xT_d, xT[kt], nc.vector),
                    (wqk_d, wqk[kt], nc.scalar),
                ):
                    raw = raw_pool.tile(dst.shape, F32, tag="raw", name=f"raw_{dram.name}_{kt}")
                    nc.sync.dma_start(out=raw, in_=dram.ap()[ksl, :])
                    if eng is nc.scalar:
                        nc.scalar.copy(out=dst, in_=raw)
                    else:
                        nc.vector.tensor_copy(dst, raw)
            for kt in range(KT):
                ksl = slice(kt * 128, (kt + 1) * 128)
                raw = raw_pool.tile([128, C], F32, tag="raw_wv", name=f"raw_wv_{kt}")
                nc.sync.dma_start(out=raw, in_=wv_d.ap()[ksl, :])
                nc.vector.tensor_copy(wv[kt], raw)

            def qk_tile(ot):
                osl = slice(ot * 128, (ot + 1) * 128)
                for nch in range(NCH):
                    nsl = slice(nch * 512, (nch + 1) * 512)
                    ps = ps_pool.tile([128, 512], F32, tag="ps_qkv", name=f"psqk_{ot}_{nch}")
                    for kt in range(KT):
                        nc.tensor.matmul(
                            ps,
                            lhsT=wqk[kt][:, osl],
                            rhs=xT[kt][:, nsl],
                            start=(kt == 0),
                            stop=(kt == KT - 1),
                        )
                    nc.vector.tensor_copy(qkT[ot][:, nsl], ps)

            # pair 0's Q and K first, then its scores+exp so ScalarE has
            # work while the rest of QKV runs on PE
            qk_tile(0)
            qk_tile(6)
            for nch in range(NCH):
                pt0[nch] = _scores_exp(nc, qkT, pt_pool, ps_s, 0, nch, f"p0_{nch}")

            for ot in (1, 7, 2, 8, 3, 9, 4, 10, 5, 11):
                qk_tile(ot)

            # V token-major with per-head ones column (65-stride layout)
            for mt in range(NT):
                msl = slice(mt * 128, (mt + 1) * 128)
                nc.vector.tensor_copy(v_sb[mt][:, :, 64:65], ones12.unsqueeze(-1))
                for o0, ow in ((0, 512), (512, 256)):
                    nh = ow // 64
                    ps = ps_pool.tile([128, 512], F32, tag="ps_qkv", name=f"psv_{mt}_{o0}")
                    for kt in range(KT):
                        nc.tensor.matmul(
                            ps[:, :ow],
                            lhsT=xT[kt][:, msl],
                            rhs=wv[kt][:, o0 : o0 + ow],
                            start=(kt == 0),
                            stop=(kt == KT - 1),
                        )
                    dst = v_sb[mt][:, o0 // 64 : o0 // 64 + nh, 0:64]
                    vsrc = ps[:, :ow].rearrange("p (h e) -> p h e", e=64)
                    nc.vector.tensor_copy(dst, vsrc)

        # ------- scope B: attention for all pairs + projection -------
        with ExitStack() as sctx:
            wp_pool = sctx.enter_context(tc.tile_pool(name="wp", bufs=1))
            raw2_pool = sctx.enter_context(tc.tile_pool(name="raw2", bufs=2))
            y_pool = sctx.enter_context(tc.tile_pool(name="y", bufs=3))
            ps_av = sctx.enter_context(tc.tile_pool(name="ps_av", bufs=2, space="PSUM"))

            wp = [wp_pool.tile([128, C], BF16, tag=f"wp{i}", name=f"wp{i}") for i in range(KT)]
            for kt in range(KT):
                raw = raw2_pool.tile([128, C], F32, tag="raw2", name=f"raw_wp_{kt}")
                nc.sync.dma_start(out=raw, in_=wp_d.ap()[kt * 128 : (kt + 1) * 128, :])
                nc.vector.tensor_copy(wp[kt], raw)

            for hp in range(H // 2):
                for nch in range(NCH):
                    tag = f"h{hp}_{nch}"
                    if hp == 0:
                        pt_tiles = pt0[nch]
                    else:
                        pt_tiles = _scores_exp(nc, qkT, pt_pool, ps_s, hp, nch, tag)
                    _av_norm(
                        nc, v_sb, attnT, pt_tiles, zs_pool, ps_av, ones_col, hp, nch, tag
                    )

            # ---------------- output projection ----------------
            for ot in range(KT):
                osl = slice(ot * 128, (ot + 1) * 128)
                for nch in range(NCH):
                    nsl = slice(nch * 512, (nch + 1) * 512)
                    ps = ps_av.tile([128, 512], F32, tag="misc", name=f"ps_y_{ot}_{nch}")
                    for kt in range(KT):
                        nc.tensor.matmul(
                            ps,
                            lhsT=wp[kt][:, osl],
                            rhs=attnT[kt][:, nsl],
                            start=(kt == 0),
                            stop=(kt == KT - 1),
                        )
                    y = y_pool.tile([128, 512], F32, tag="y")
                    nc.scalar.activation(y, ps, AF.Identity, bias=b_sb[:, ot : ot + 1])
                    nc.sync.dma_start(out=out_d.ap()[osl, nsl], in_=y)


def _get_nc():
    if "nc" not in _CACHE:
        _CACHE["nc"] = build()
    return _CACHE["nc"]


def kernel(x, w_qkv, w_proj, b_proj, _trace=False):
    x = np.asarray(x, dtype=np.float32)
    w_qkv = np.asarray(w_qkv, dtype=np.float32)
    w_proj = np.asarray(w_proj, dtype=np.float32)
    b_proj = np.asarray(b_proj, dtype=np.float32)

    wq = w_qkv[0:C] * np.float32(SCALE)
    wk = w_qkv[C : 2 * C]
    wv = w_qkv[2 * C : 3 * C]
    w_qkT = np.ascontiguousarray(np.concatenate([wq, wk], axis=0).T)  # [C, 2C]
    w_vT = np.ascontiguousarray(wv.T)  # [C, C]
    w_pT = np.ascontiguousarray(w_proj.T)  # [C, C]
    b_p = np.ascontiguousarray(b_proj.reshape(C, 1))

    in_maps = []
    for i in range(B):
        in_maps.append(
            {
                "xT": np.ascontiguousarray(x[i].T),  # [C, N]
                "w_qkT": w_qkT,
                "w_vT": w_vT,
                "w_pT": w_pT,
                "b_p": b_p,
            }
        )

    nc = _get_nc()
    res = run_bass_kernel_spmd(nc, in_maps, core_ids=list(range(B)), trace=_trace)
    _CACHE["last_result"] = res

    out = np.empty((B, N, C), dtype=np.float32)
    for i in range(B):
        out[i] = res.results[i]["out"].T
    return out
